# revision 23
# baseline (speedup 1.0000x reference)
"""Deformable-conv Bass kernel for Trainium2, SPMD over 8 NeuronCores.

Sharding: data-parallel over (batch n, image half). Each core computes a
[O, 64, 128] half-sample. Inside a core:
  1. offset conv (3x3, C=64 -> 18) as 9 PSUM-accumulated matmuls per chunk
  2. PE-transpose offsets to pixel-major [w, h, 18]
  3. bilinear coeffs + int16 gather indices on DVE (floor via mod)
  4. dma_gather of 2-pixel row pairs from a zero-padded pixel-major copy
     of x in DRAM (padding implements the reference's out-of-bounds mask)
  5. corner-weighted sum on DVE (coeff broadcast along channels)
  6. PE-transpose the weighted im2col, 5-chunk PSUM-accumulated GEMM with
     w_def, bias add via ScalarE.
"""
import numpy as np

import concourse.bass as bass
import concourse.mybir as mybir
from concourse import bacc
from concourse.bass_utils import run_bass_kernel_spmd
from concourse.masks import make_identity
from concourse.tile import TileContext

N, C, H, W, O = 4, 64, 128, 128, 64
K, KK, PAD = 3, 9, 4
HP = H + 2 * PAD            # 136 padded rows/cols
NPIX_P = HP * HP            # 18496
NWIN = NPIX_P + 4           # patch-table rows (2x2 pixel patches), small slack
HH = H // 2                 # 64 output rows per core
NCORES = 8
CH = 16                     # h-chunk rows in the tap loop
NCH = HH // CH              # 2 chunks
KC = KK * C                 # 576 contraction size
KCP = 640                   # padded to 5*128
NJ = KCP // 128             # 5 GEMM chunks

F32 = mybir.dt.float32
I16 = mybir.dt.int16
I32 = mybir.dt.int32
ALU = mybir.AluOpType
ACT_COPY = mybir.ActivationFunctionType.Copy
ACT_IDENT = mybir.ActivationFunctionType.Identity

_CACHE: dict = {}
DEBUG_TAPS = False  # add intermediate-dump outputs
SKIP_GATHER = False


def _sq(ap):
    """Drop size-1 free dims (keep partition dim) so DMA AP balancing works."""
    dims = [ap.ap[0]] + [d for d in ap.ap[1:] if d[1] != 1]
    if len(dims) == 1:
        dims.append([1, 1])
    return bass.AP(ap.tensor, ap.offset, dims)


def _build():
    nc = bacc.Bacc("TRN2", target_bir_lowering=False, debug=True)

    xp_d = nc.dram_tensor("xp", [NWIN, 4 * C], F32, kind="ExternalInput")
    xc_d = nc.dram_tensor("xc", [C, (HH + 2) * (W + 2)], F32, kind="ExternalInput")
    woff_d = nc.dram_tensor("woff", [C, KK * 18], F32, kind="ExternalInput")
    wdef_d = nc.dram_tensor("wdef", [128, NJ * O], F32, kind="ExternalInput")
    boff_d = nc.dram_tensor("boff", [18, 1], F32, kind="ExternalInput")
    bdef_d = nc.dram_tensor("bdef", [O, 1], F32, kind="ExternalInput")
    tby_d = nc.dram_tensor("tby", [W, HH * KK], F32, kind="ExternalInput")
    tbx_d = nc.dram_tensor("tbx", [W, HH * KK], F32, kind="ExternalInput")
    out_d = nc.dram_tensor("out", [O, HH * W], F32, kind="ExternalOutput")
    if DEBUG_TAPS:
        dbg_offt = nc.dram_tensor("dbg_offt", [W, HH * 18], F32, kind="ExternalOutput")
        dbg_a00 = nc.dram_tensor("dbg_a00", [W, HH * KK], F32, kind="ExternalOutput")
        dbg_idx = nc.dram_tensor("dbg_idx", [W, KK * HH], I32, kind="ExternalOutput")
        dbg_v0 = nc.dram_tensor("dbg_v0", [128, CH * 2 * C], F32, kind="ExternalOutput")
        dbg_s = nc.dram_tensor("dbg_s", [W, CH * KCP], F32, kind="ExternalOutput")


    with TileContext(nc) as tc:
        with (
            tc.tile_pool(name="const", bufs=1) as cpool,
            tc.tile_pool(name="work", bufs=2) as wpool,
        ):
            WOFF = cpool.tile([C, KK, 18], F32)
            nc.sync.dma_start(WOFF[:], woff_d.ap().rearrange("c (k o) -> c k o", k=KK))
            WDEF = cpool.tile([128, NJ, O], F32)
            nc.sync.dma_start(WDEF[:], wdef_d.ap().rearrange("p (j o) -> p j o", j=NJ))
            BOFF = cpool.tile([18, 1], F32)
            nc.sync.dma_start(BOFF[:], boff_d[:])
            BDEF = cpool.tile([O, 1], F32)
            nc.sync.dma_start(BDEF[:], bdef_d[:])
            IDENT = cpool.tile([128, 128], F32)
            make_identity(nc, IDENT[:])

            A00 = cpool.tile([W, HH, KK], F32)
            A01 = cpool.tile([W, HH, KK], F32)
            A10 = cpool.tile([W, HH, KK], F32)
            A11 = cpool.tile([W, HH, KK], F32)
            IDX0 = cpool.tile([W, KK, HH], I32)  # k-major: [:, k, h] is per-partition

            with tc.tile_pool(name="coef", bufs=1) as kpool:
                XC = kpool.tile([C, HH + 2, W + 2], F32)
                nc.sync.dma_start(
                    XC[:], xc_d.ap().rearrange("c (h w) -> c h w", w=W + 2)
                )
                TBY = kpool.tile([W, HH, KK], F32)
                nc.sync.dma_start(
                    TBY[:], tby_d.ap().rearrange("w (h k) -> w h k", k=KK)
                )
                TBX = kpool.tile([W, HH, KK], F32)
                nc.sync.dma_start(
                    TBX[:], tbx_d.ap().rearrange("w (h k) -> w h k", k=KK)
                )
                OFFT = kpool.tile([W, HH, 18], F32)  # (w, h, ch)

                # ---- offset conv + transpose to pixel-major ----
                with (
                    tc.tile_pool(name="psA", bufs=2, space="PSUM") as psA,
                    tc.tile_pool(name="psB", bufs=2, space="PSUM") as psB,
                ):
                    for chunk in range(16):       # 4 output rows per chunk
                        hh = 4 * chunk
                        offp = psA.tile([18, 512], F32)
                        for k in range(KK):
                            ki, kj = k // K, k % K
                            rhs = XC[:, hh + ki:hh + ki + 4, kj:kj + W]
                            nc.tensor.matmul(
                                offp[:], WOFF[:, k, :], rhs,
                                start=(k == 0), stop=(k == KK - 1),
                            )
                        offs = wpool.tile([18, 512], F32, tag="offs")
                        nc.scalar.activation(offs[:], offp[:], ACT_IDENT, bias=BOFF[:])
                        tp = psB.tile([128, 4, 18], F32)
                        for j in range(4):
                            nc.tensor.transpose(
                                out=tp[:, j, :], in_=offs[:, j * W:(j + 1) * W],
                                identity=IDENT[:18, :18],
                            )
                        nc.vector.tensor_copy(OFFT[:, hh:hh + 4, :], tp[:])

                # ---- bilinear coeffs + gather indices ----
                dy = OFFT[:, :, 0::2]
                dx = OFFT[:, :, 1::2]
                PY = kpool.tile([W, HH, KK], F32)
                nc.vector.tensor_tensor(PY[:], TBY[:], dy, ALU.add)
                PX = kpool.tile([W, HH, KK], F32)
                nc.vector.tensor_tensor(PX[:], TBX[:], dx, ALU.add)
                # floor(p) = round(p) - (round(p) > p); DVE f32->i32 cast rounds
                RI = kpool.tile([W, HH, KK], I32)
                RF = kpool.tile([W, HH, KK], F32)
                G = kpool.tile([W, HH, KK], F32)
                Y0 = kpool.tile([W, HH, KK], F32)
                WY = kpool.tile([W, HH, KK], F32)
                X0 = kpool.tile([W, HH, KK], F32)
                WX = kpool.tile([W, HH, KK], F32)
                nc.vector.tensor_copy(RI[:], PY[:])
                nc.vector.tensor_copy(RF[:], RI[:])
                nc.vector.tensor_tensor(G[:], RF[:], PY[:], ALU.is_gt)
                nc.vector.tensor_tensor(Y0[:], RF[:], G[:], ALU.subtract)
                nc.vector.tensor_tensor(WY[:], PY[:], Y0[:], ALU.subtract)
                nc.vector.tensor_copy(RI[:], PX[:])
                nc.vector.tensor_copy(RF[:], RI[:])
                nc.vector.tensor_tensor(G[:], RF[:], PX[:], ALU.is_gt)
                nc.vector.tensor_tensor(X0[:], RF[:], G[:], ALU.subtract)
                nc.vector.tensor_tensor(WX[:], PX[:], X0[:], ALU.subtract)
                CY = kpool.tile([W, HH, KK], F32)
                nc.vector.tensor_scalar(CY[:], WY[:], -1.0, 1.0, ALU.mult, ALU.add)
                CX = kpool.tile([W, HH, KK], F32)
                nc.vector.tensor_scalar(CX[:], WX[:], -1.0, 1.0, ALU.mult, ALU.add)
                nc.vector.tensor_tensor(A00[:], CY[:], CX[:], ALU.mult)
                nc.vector.tensor_tensor(A01[:], CY[:], WX[:], ALU.mult)
                nc.vector.tensor_tensor(A10[:], WY[:], CX[:], ALU.mult)
                nc.vector.tensor_tensor(A11[:], WY[:], WX[:], ALU.mult)
                IDXF = kpool.tile([W, HH, KK], F32)
                nc.vector.tensor_scalar(IDXF[:], Y0[:], float(HP), None, ALU.mult)
                nc.vector.tensor_tensor(IDXF[:], IDXF[:], X0[:], ALU.add)
                nc.vector.tensor_copy(IDX0[:].transpose([0, 2, 1]), IDXF[:])
                if DEBUG_TAPS:
                    nc.sync.dma_start(dbg_offt[:], OFFT[:])
                    nc.sync.dma_start(dbg_a00[:], A00[:])
                    nc.sync.dma_start(dbg_idx[:], IDX0[:])

            # ---- tap loop: gather, weight, transpose, GEMM ----
            OUT = cpool.tile([O, HH * W], F32)
            with (
                tc.tile_pool(name="vpool", bufs=4) as vpool,
                tc.tile_pool(name="spool", bufs=1) as spool,
                tc.tile_pool(name="psC", bufs=2, space="PSUM") as psC,
                tc.tile_pool(name="psD", bufs=2, space="PSUM") as psD,
            ):
                for hc in range(NCH):
                    S = spool.tile([W, CH, KCP], F32)
                    nc.vector.memset(S[:, :, KC:], 0.0)
                    hsl = slice(hc * CH, (hc + 1) * CH)
                    for k in range(KK):
                        V = vpool.tile([128, CH, 4 * C], F32, tag="v0")
                        if SKIP_GATHER:
                            nc.vector.memset(V[:], 0.0)
                        else:
                            for hl in range(CH):
                                nc.gpsimd.indirect_dma_start(
                                    out=V[:, hl, :],
                                    out_offset=None,
                                    in_=xp_d[:],
                                    in_offset=bass.IndirectOffsetOnAxis(
                                        ap=IDX0[:, k, hc * CH + hl:hc * CH + hl + 1],
                                        axis=0,
                                    ),
                                )
                        V0 = V
                        V1 = V
                        sk = S[:, :, k * C:(k + 1) * C]

                        def bc(a):
                            return a[:, hsl, k:k + 1].to_broadcast([W, CH, C])

                        TMP = wpool.tile([W, CH, C], F32, tag="wtmp")
                        nc.vector.tensor_tensor(sk, V[:, :, 0:C], bc(A00), ALU.mult)
                        nc.vector.tensor_tensor(TMP[:], V[:, :, C:2 * C], bc(A01), ALU.mult)
                        nc.vector.tensor_tensor(sk, sk, TMP[:], ALU.add)
                        if DEBUG_TAPS and hc == 0 and k == 0:
                            nc.sync.dma_start(dbg_v0[:], V[:, :, 0:2 * C])
                        nc.vector.tensor_tensor(TMP[:], V[:, :, 2 * C:3 * C], bc(A10), ALU.mult)
                        nc.vector.tensor_tensor(sk, sk, TMP[:], ALU.add)
                        nc.vector.tensor_tensor(TMP[:], V[:, :, 3 * C:4 * C], bc(A11), ALU.mult)
                        nc.vector.tensor_tensor(sk, sk, TMP[:], ALU.add)

                    if DEBUG_TAPS and hc == 0:
                        nc.sync.dma_start(dbg_s[:], S[:])
                    for h in range(CH):
                        stp = psC.tile([128, NJ, 128], F32)
                        for j in range(NJ):
                            nc.tensor.transpose(
                                out=stp[:, j, :],
                                in_=S[:, h, j * 128:(j + 1) * 128],
                                identity=IDENT[:],
                            )
                        scp = wpool.tile([128, NJ, 128], F32, tag="scp")
                        nc.scalar.copy(scp[:], stp[:])
                        outp = psD.tile([O, W], F32)
                        for j in range(NJ):
                            nc.tensor.matmul(
                                outp[:], WDEF[:, j, :], scp[:, j, :],
                                start=(j == 0), stop=(j == NJ - 1),
                            )
                        hg = hc * CH + h
                        nc.scalar.activation(
                            OUT[:, hg * W:(hg + 1) * W], outp[:],
                            ACT_IDENT, bias=BDEF[:],
                        )
            nc.sync.dma_start(out_d[:], OUT[:])

    nc.compile()
    return nc


def get_nc():
    if "nc" not in _CACHE:
        _CACHE["nc"] = _build()
    return _CACHE["nc"]


def make_core_inputs(x, w_off, b_off, w_def, b_def):
    """Host-side shard prep: layout/pad transforms only."""
    x = np.ascontiguousarray(x, np.float32)
    # w_off [18, C, 3, 3] -> lhsT per tap: woff[c, k, o18]
    woff = np.ascontiguousarray(
        np.transpose(w_off.reshape(2 * KK, C, K * K), (1, 2, 0)).astype(np.float32)
    ).reshape(C, KK * 18)
    wdef_kc = np.zeros((KCP, O), np.float32)
    wdef_kc[:KC] = w_def.reshape(O, C, KK).transpose(2, 1, 0).reshape(KC, O)
    wdef = np.ascontiguousarray(
        wdef_kc.reshape(NJ, 128, O).transpose(1, 0, 2)
    ).reshape(128, NJ * O)
    boff = b_off.reshape(18, 1).astype(np.float32)
    bdef = b_def.reshape(O, 1).astype(np.float32)

    ki = (np.arange(KK) // K).astype(np.float32)
    kj = (np.arange(KK) % K).astype(np.float32)
    wloc = np.arange(W, dtype=np.float32)
    hloc = np.arange(HH, dtype=np.float32)
    tbx = np.broadcast_to(
        wloc[:, None, None] + kj[None, None, :] - 1 + PAD, (W, HH, KK)
    ).astype(np.float32).reshape(W, HH * KK)

    in_maps = []
    for core in range(NCORES):
        n, half = core // 2, core % 2
        h0 = half * HH
        xpim = np.pad(x[n].transpose(1, 2, 0),
                      ((PAD, PAD + 1), (PAD, PAD + 1), (0, 0)))  # [HP+1, HP+1, C]
        patch = np.concatenate(
            [xpim[:HP, :HP], xpim[:HP, 1:HP + 1],
             xpim[1:HP + 1, :HP], xpim[1:HP + 1, 1:HP + 1]], axis=2)
        xpad = np.zeros((NWIN, 4 * C), np.float32)
        xpad[:NPIX_P] = patch.reshape(NPIX_P, 4 * C)
        pad1 = np.pad(x[n], ((0, 0), (1, 1), (1, 1)))
        xc = np.ascontiguousarray(pad1[:, h0:h0 + HH + 2, :]).reshape(
            C, (HH + 2) * (W + 2)
        )
        tby = np.broadcast_to(
            (h0 + hloc[:, None]) + ki[None, :] - 1 + PAD, (W, HH, KK)
        ).astype(np.float32).reshape(W, HH * KK)
        in_maps.append({
            "xp": xpad, "xc": xc, "woff": woff, "wdef": wdef,
            "boff": boff, "bdef": bdef, "tby": tby, "tbx": tbx,
        })
    return in_maps


def assemble(results):
    full = np.zeros((N, O, H, W), np.float32)
    for core in range(NCORES):
        n, half = core // 2, core % 2
        h0 = half * HH
        full[n, :, h0:h0 + HH, :] = results[core]["out"].reshape(O, HH, W)
    return full


def kernel(x, w_off, b_off, w_def, b_def):
    nc = get_nc()
    in_maps = make_core_inputs(x, w_off, b_off, w_def, b_def)
    res = run_bass_kernel_spmd(nc, in_maps, list(range(NCORES)))
    return assemble(res.results)
